# revision 41
# baseline (speedup 1.0000x reference)
"""Trainium2 Bass kernel for nn_ClassifyMCLoss (loss_fn).

Computes (iou_loss, cls_loss) of the reference:
  - spatial max over pred_mask_prob (bs,ch,256,256)  <- dominant, memory-bound
  - gathers via map_indices, smooth-l1 on iou preds, weighted CE on logits
  - weighted sums / (sum of weights + 1e-4)

Sharding: data-parallel over batch. 8 cores x 2 batches each; the 2*64
(batch, channel) rows of a shard map exactly onto 128 SBUF partitions.
Each core emits 3 partial sums; the host reduces across cores and divides.

Raw Bass (no Tile): this toolchain's walrus rejects DMA instructions with
more than one sync-wait, which Tile's scheduler emits for recycled stream
buffers. Explicit semaphores + standalone sequencer waits keep every DMA at
<=1 wait and avoid Tile's kernel-tail drain overhead.

Engine split:
  SP (sync)  : even streaming chunks of pred_mask_prob, the packed-blob
               load wedged after chunk 0, and the result writeback
  ACT        : odd streaming chunks, with exp/ln of the log-softmax wedged
               between two of them mid-stream
  Pool       : unused (its SWDGE preamble exists regardless; keeping it
               empty avoids queueing small DMAs behind stream transfers)
  DVE        : chunk row-max reduces + all elementwise math
  PE         : one-hot gather matmuls + weighted-sum matmuls

All small inputs (logits, iou scores, targets, map indices, random values,
iota tables) are packed host-side into a single [128, W] f32 blob (int32
fields are shipped as raw bits and bitcast on device) so exactly one small
DMA is issued -- many small DMAs would each queue behind an in-flight
multi-MiB streaming transfer and starve the gather pipeline.
The one-hot gather matrices are built on device from index COLUMNS: the
transposed one-hots ohT[j,c] = (idx[j]+64*b == c) come from an is_equal
against an iota table, and the c-major orientation needed as matmul lhsT
is produced by PE-transposing ohT against an on-device identity (keeps
the blob small; a host-replicated row form cost 2x blob transfer time).

Latency shaping: chunk sizes taper into the tail so the final row-max
reduce after the last DMA byte is ~1.2us, and everything not depending on
the spatial max is precomputed in DVE/ACT/PE idle slack while streaming.
The remove-mask is algebraically split --
  wght = wcls*im2 + wcls*(1-im2)*[max_prob[pj] >= thr]
-- so the mp-independent base sums B and the per-row threshold terms
R = onehot_pj^T @ (v * wcls * (1-im2)) are both computed mid-stream; the
tail is only: last small reduce -> row-max -> threshold -> S2 = th^T @ R
(tiny matmul) -> res = B + S2 -> 12-byte writeback.
"""

import numpy as np

# problem constants (hardcoded per spec nn_ClassifyMCLoss_90726889161276)
BS, CH, C, H, W = 16, 64, 81, 256, 256
N_CORES = 8
BPC = BS // N_CORES            # batches per core = 2
ROWS = BPC * CH                # 128 rows per core
SPATIAL = H * W                # 65536
K = CH - 1                     # 63 (loop range FG_STCH..ch)
NJ = BPC * K                   # 126 (b,k) pairs per core
NBUF = 3                       # stream buffers

# tapered chunk sizes (columns of the [128, 65536] view); reduce(k) must end
# before dma(k+1) does, so sizes shrink by ~0.75x into the tail.
CHUNKS = [8192] * 4 + [4096] * 6 + [2816, 2304, 1792, 1280]
assert sum(CHUNKS) == SPATIAL
NCHUNK = len(CHUNKS)

# blob column layout ([128, WBLOB] f32; int32 fields shipped as raw bits)
COL_IOUSC = 0                  # [128] iou_scores
COL_LG = 1                     # [128, 81] cls_logits
COL_TGT = 1 + C                # [128] target_ids (i32 bits)
COL_IOU = COL_TGT + 1          # [126] map_ious[:, 1:]
COL_RND = COL_IOU + 1          # [126] rand_vals[:, 1:]
COL_IOTA = COL_RND + 1         # [128] partition index (f32)
COL_IOTA2D = COL_IOTA + 1      # [128, 128] free-index iota (f32)
COL_PJC = COL_IOTA2D + ROWS    # [126] pj as a column (i32 bits)
COL_GJC = COL_PJC + 1          # [126] gj as a column (i32 bits)
COL_BADJ = COL_GJC + 1         # [126] flat-row bias: 0 (b=0) / 64 (b=1)
WBLOB = COL_BADJ + 1

_CACHE = {}


def _build_nc():
    from contextlib import ExitStack

    import concourse.bass as bass
    from concourse import mybir

    f32 = mybir.dt.float32
    i32 = mybir.dt.int32
    Alu = mybir.AluOpType
    Act = mybir.ActivationFunctionType
    AX = mybir.AxisListType.X

    nc = bass.Bass("TRN2", target_bir_lowering=False, debug=False)

    pmp = nc.dram_tensor("pmp", [ROWS, SPATIAL], f32, kind="ExternalInput").ap()
    blob = nc.dram_tensor("blob", [ROWS, WBLOB], f32, kind="ExternalInput").ap()
    out = nc.dram_tensor("out", [1, 3], f32, kind="ExternalOutput").ap()

    # chunk start offsets
    offs = [0]
    for s in CHUNKS:
        offs.append(offs[-1] + s)

    with ExitStack() as ctx:
        def sb(name, shape, dtype=f32):
            return ctx.enter_context(nc.sbuf_tensor(name, shape, dtype))

        def psum(name, shape):
            return ctx.enter_context(nc.psum_tensor(name, shape, f32))

        def sem(name):
            return ctx.enter_context(nc.semaphore(name))

        t_bufs = [sb(f"t{i}", [ROWS, max(CHUNKS)]) for i in range(NBUF)]
        partial = sb("partial", [ROWS, NCHUNK])
        maxp = sb("maxp", [ROWS, 1])
        bsb = sb("bsb", [ROWS, WBLOB])             # the packed blob
        tgt_f = sb("tgt_f", [ROWS, 1])
        oh_pj = sb("oh_pj", [ROWS, NJ])
        oh_gj = sb("oh_gj", [ROWS, NJ])
        pjc_f = sb("pjc_f", [NJ, 1])
        gjc_f = sb("gjc_f", [NJ, 1])
        ohTp_s = sb("ohTp_s", [NJ, ROWS])
        ohTg_s = sb("ohTg_s", [NJ, ROWS])
        ident = sb("ident", [NJ, NJ])
        cls_f = sb("cls_f", [NJ, 1])
        im2 = sb("im2", [NJ, 1])
        m2 = sb("m2", [NJ, 1])
        wcls = sb("wcls", [NJ, 1])
        wb = sb("wb", [NJ, 1])
        ws = sb("ws", [NJ, 1])
        ones126 = sb("ones126", [NJ, 1])
        yv = sb("yv", [NJ, 1])
        ny = sb("ny", [NJ, 1])
        ltm = sb("ltm", [NJ, 1])
        a5 = sb("a5", [NJ, 1])
        bl = sb("bl", [NJ, 1])
        el = sb("el", [NJ, 1])
        rmax = sb("rmax", [NJ, 1])
        nrmax = sb("nrmax", [NJ, 1])
        ex = sb("ex", [NJ, C])
        sm = sb("sm", [NJ, 1])
        ls = sb("ls", [NJ, 1])
        ohc = sb("ohc", [NJ, C])
        ohs = sb("ohs", [NJ, C])
        lgc = sb("lgc", [NJ, 1])
        ce = sb("ce", [NJ, 1])
        qs = sb("qs", [NJ, 3])
        qb = sb("qb", [NJ, 3])
        r_s = sb("r_s", [ROWS, 3])
        th = sb("th", [ROWS, 1])
        res_b = sb("res_b", [1, 3])
        res = sb("res", [1, 3])

        # blob views
        data2 = bsb[:, COL_IOUSC:COL_IOUSC + 1 + C]   # [iou_sc | logits]
        tgt_i = bsb[:, COL_TGT:COL_TGT + 1].bitcast(i32)
        iou_t = bsb[0:NJ, COL_IOU:COL_IOU + 1]
        rnd_t = bsb[0:NJ, COL_RND:COL_RND + 1]
        iota_f = bsb[:, COL_IOTA:COL_IOTA + 1]
        iota2d = bsb[0:NJ, COL_IOTA2D:COL_IOTA2D + ROWS]
        iotaC_f = bsb[0:NJ, COL_IOTA2D:COL_IOTA2D + C]
        pjc_i = bsb[0:NJ, COL_PJC:COL_PJC + 1].bitcast(i32)
        gjc_i = bsb[0:NJ, COL_GJC:COL_GJC + 1].bitcast(i32)
        badj = bsb[0:NJ, COL_BADJ:COL_BADJ + 1]

        tr_p = psum("tr_p", [ROWS, NJ])            # transposed pj one-hot
        tr_g = psum("tr_g", [ROWS, NJ])            # transposed gj one-hot
        pg1 = psum("pg1", [NJ, 1 + C])             # [piou | logits] gather
        pt = psum("pt", [NJ, 1])                   # target_ids gather
        r_p = psum("r_p", [ROWS, 3])               # per-row threshold terms
        b_p = psum("b_p", [1, 3])                  # mp-independent base sums
        s2_p = psum("s2_p", [1, 3])                # threshold-term sums

        # per-slot sems: completions are unordered within an engine, so each
        # stream buffer gets its own DMA-completion and reduce-completion sem
        # to keep every wait value unambiguous.
        bigdS = [sem(f"bigd{i}") for i in range(NBUF)]   # stream DMA done (x16)
        redS = [sem(f"red{i}") for i in range(NBUF)]     # slot reduce done
        dio = sem("dio")            # blob DMA completion (x16)
        outd = sem("outd")          # output DMA completion (x16)
        ohT_done = sem("ohT_done")
        oh_done = sem("oh_done")
        q_done = sem("q_done")
        th_done = sem("th_done")
        nr_done = sem("nr_done")
        act_s = sem("act_s")
        pe_s = sem("pe_s")
        fin = sem("fin")

        blk = ctx.enter_context(nc.Block())

        # streaming chunks alternate between the two HWDGE queues (SP and
        # ACT) so each queue's per-DMA trigger/completion overhead hides
        # behind the other queue's transfer.
        def stream_chunks(eng, parity, start=0, stop=None):
            for j in range(start, NCHUNK if stop is None else stop):
                if j % 2 != parity:
                    continue
                s = j % NBUF
                if j >= NBUF:
                    eng.wait_ge(redS[s], j // NBUF)
                eng.dma_start(
                    out=t_bufs[s][:, 0:CHUNKS[j]],
                    in_=pmp[:, offs[j]:offs[j + 1]],
                ).then_inc(bigdS[s], 16)

        # -------- SP: even stream chunks with the blob load wedged after --
        # -------- chunk 0 (SP's queue has no mid-stream waits, and an ------
        # -------- unused Pool engine keeps its SWDGE preamble from gating --
        # -------- the all-engine start barrier), then result writeback -----
        @blk.sync
        def _(sp):
            stream_chunks(sp, 0, stop=1)
            sp.dma_start(out=bsb[:], in_=blob).then_inc(dio, 16)
            stream_chunks(sp, 0, start=1)
            sp.wait_ge(fin, 1)
            sp.dma_start(out=out, in_=res[:]).then_inc(outd, 16)
            sp.wait_ge(outd, 16)

        # -------- DVE: reduces + elementwise math --------
        @blk.vector
        def _(v):
            slot_use = [0] * NBUF

            def reduce_chunk(j):
                s = j % NBUF
                slot_use[s] += 1
                v.wait_ge(bigdS[s], 16 * slot_use[s])
                v.reduce_max(out=partial[:, j:j + 1],
                             in_=t_bufs[s][:, 0:CHUNKS[j]],
                             axis=AX).then_inc(redS[s], 1)

            reduce_chunk(0)
            # P0: cast target ids + index columns, build the identity
            v.wait_ge(dio, 16)
            v.tensor_copy(out=tgt_f[:], in_=tgt_i)
            v.memset(ones126[:], 1.0)
            v.tensor_copy(out=pjc_f[:], in_=pjc_i)
            v.tensor_copy(out=gjc_f[:], in_=gjc_i)
            v.tensor_scalar(out=ident[:], in0=iota2d[:, 0:NJ],
                            scalar1=iota_f[0:NJ, :], scalar2=None,
                            op0=Alu.is_equal)
            v.drain()
            # adjust to flat (b*64 + idx) rows
            v.tensor_tensor(out=pjc_f[:], in0=pjc_f[:], in1=badj, op=Alu.add)
            v.tensor_tensor(out=gjc_f[:], in0=gjc_f[:], in1=badj, op=Alu.add)
            v.drain()
            # P1: transposed one-hots [j, c] = (idx[j] == c)
            v.tensor_scalar(out=ohTp_s[:], in0=iota2d, scalar1=pjc_f[:],
                            scalar2=None, op0=Alu.is_equal)
            v.tensor_scalar(out=ohTg_s[:], in0=iota2d, scalar1=gjc_f[:],
                            scalar2=None, op0=Alu.is_equal)
            v.drain().then_inc(ohT_done, 1)
            # P2: stage PE-transposed one-hots into SBUF for the gathers
            v.wait_ge(pe_s, 2)
            v.tensor_copy(out=oh_pj[:], in_=tr_p[:])
            v.tensor_copy(out=oh_gj[:], in_=tr_g[:])
            v.drain().then_inc(oh_done, 1)
            reduce_chunk(1)
            # stage A: independents off pg1 + loads
            v.wait_ge(pe_s, 4)
            v.tensor_copy(out=cls_f[:], in_=pt[:])
            v.tensor_scalar(out=im2[:], in0=rnd_t, scalar1=0.9,
                            scalar2=None, op0=Alu.is_ge)      # 1 - (rnd<.9)
            v.tensor_scalar(out=m2[:], in0=rnd_t, scalar1=0.9,
                            scalar2=None, op0=Alu.is_lt)      # rnd<.9
            v.tensor_scalar(out=wcls[:], in0=iou_t, scalar1=0.2,
                            scalar2=1.0, op0=Alu.is_ge, op1=Alu.add)
            v.tensor_tensor(out=yv[:], in0=pg1[:, 0:1], in1=iou_t,
                            op=Alu.subtract)
            v.reduce_max(out=rmax[:], in_=pg1[:, 1:1 + C], axis=AX)
            v.drain()
            reduce_chunk(2)
            # stage B
            v.scalar_tensor_tensor(out=ny[:], in0=yv[:], scalar=-1.0,
                                   in1=yv[:], op0=Alu.mult, op1=Alu.max)  # |y|
            v.tensor_scalar(out=nrmax[:], in0=rmax[:], scalar1=-1.0,
                            scalar2=None, op0=Alu.mult)
            v.tensor_scalar(out=ohc[:], in0=iotaC_f, scalar1=cls_f[:],
                            scalar2=None, op0=Alu.is_equal)
            v.drain().then_inc(nr_done, 1)
            reduce_chunk(3)
            # stage C
            v.tensor_scalar(out=ltm[:], in0=ny[:], scalar1=0.1, scalar2=None,
                            op0=Alu.is_lt)
            v.scalar_tensor_tensor(out=a5[:], in0=ny[:], scalar=5.0,
                                   in1=ny[:], op0=Alu.mult, op1=Alu.mult)  # 5y^2
            v.tensor_scalar(out=bl[:], in0=ny[:], scalar1=-0.05, scalar2=None,
                            op0=Alu.add)
            v.tensor_tensor(out=ohs[:], in0=pg1[:, 1:1 + C], in1=ohc[:],
                            op=Alu.mult)
            v.drain()
            reduce_chunk(4)
            # stage D
            v.tensor_tensor(out=el[:], in0=a5[:], in1=bl[:], op=Alu.subtract)
            v.reduce_sum(out=lgc[:], in_=ohs[:], axis=AX)   # logits[cls]
            v.tensor_tensor(out=wb[:], in0=wcls[:], in1=im2[:], op=Alu.mult)
            v.tensor_tensor(out=ws[:], in0=wcls[:], in1=m2[:], op=Alu.mult)
            v.drain()
            reduce_chunk(5)
            # stage E
            v.scalar_tensor_tensor(out=el[:], in0=el[:], scalar=ltm[:],
                                   in1=bl[:], op0=Alu.mult, op1=Alu.add)
            v.drain()
            reduce_chunk(6)
            # stage F: ce once ACT's mid-stream activations finish
            v.wait_ge(act_s, 1)
            # ce = (ls + rmax) - logits[cls]; ACT already folded rmax into ls
            v.scalar_tensor_tensor(out=ce[:], in0=lgc[:], scalar=-1.0,
                                   in1=ls[:], op0=Alu.mult, op1=Alu.add)
            v.drain()
            reduce_chunk(7)
            # stage G: weighted terms for both branches of the remove mask
            v.tensor_tensor(out=qs[:, 0:1], in0=el[:], in1=ws[:], op=Alu.mult)
            v.tensor_tensor(out=qs[:, 1:2], in0=ce[:], in1=ws[:], op=Alu.mult)
            v.tensor_copy(out=qs[:, 2:3], in_=ws[:])
            v.tensor_tensor(out=qb[:, 0:1], in0=el[:], in1=wb[:], op=Alu.mult)
            v.tensor_tensor(out=qb[:, 1:2], in0=ce[:], in1=wb[:], op=Alu.mult)
            v.tensor_copy(out=qb[:, 2:3], in_=wb[:])
            v.drain().then_inc(q_done, 1)
            reduce_chunk(8)
            # stage H: stage the threshold-term matrix + base sums in SBUF
            v.wait_ge(pe_s, 6)
            v.tensor_copy(out=r_s[:], in_=r_p[:])
            v.tensor_copy(out=res_b[:], in_=b_p[:])
            v.drain()
            for j in range(9, NCHUNK):
                reduce_chunk(j)
            v.drain()                # flush partial[] writes
            v.reduce_max(out=maxp[:], in_=partial[:], axis=AX)
            v.drain()

            # ---- tail: threshold -> S2 matmul -> add base ----
            v.tensor_scalar(out=th[:], in0=maxp[:], scalar1=0.1,
                            scalar2=None, op0=Alu.is_ge)      # 1 - (mp<.1)
            v.drain().then_inc(th_done, 1)
            v.wait_ge(pe_s, 7)
            v.tensor_tensor(out=res[:], in0=res_b[:], in1=s2_p[:], op=Alu.add)
            v.drain().then_inc(fin, 1)

        # -------- PE: gather matmuls + final sum --------
        @blk.tensor
        def _(pe):
            pe.wait_ge(ohT_done, 1)
            pe.matmul(tr_p[:], lhsT=ohTp_s[:], rhs=ident[:], start=True,
                      stop=True).then_inc(pe_s, 1)
            pe.matmul(tr_g[:], lhsT=ohTg_s[:], rhs=ident[:], start=True,
                      stop=True).then_inc(pe_s, 1)
            pe.wait_ge(oh_done, 1)
            pe.matmul(pg1[:], lhsT=oh_pj[:], rhs=data2, start=True,
                      stop=True).then_inc(pe_s, 1)
            pe.matmul(pt[:], lhsT=oh_gj[:], rhs=tgt_f[:], start=True,
                      stop=True).then_inc(pe_s, 1)
            pe.wait_ge(q_done, 1)
            pe.matmul(r_p[:], lhsT=ohTp_s[:], rhs=qs[:], start=True,
                      stop=True).then_inc(pe_s, 1)
            pe.matmul(b_p[:], lhsT=ones126[:], rhs=qb[:], start=True,
                      stop=True).then_inc(pe_s, 1)
            pe.wait_ge(th_done, 1)
            pe.matmul(s2_p[:], lhsT=th[:], rhs=r_s[:], start=True,
                      stop=True).then_inc(pe_s, 1)

        # -------- ACT: odd stream chunks with mid-stream exp/ln ------------
        @blk.scalar
        def _(a):
            stream_chunks(a, 1, stop=7)
            a.wait_ge(pe_s, 4)
            a.wait_ge(nr_done, 1)
            a.activation(out=ex[:], in_=pg1[:, 1:1 + C], func=Act.Exp,
                         bias=nrmax[:], scale=1.0, accum_out=sm[:])
            a.drain()
            a.activation(out=ls[:], in_=sm[:], func=Act.Ln)
            a.drain()
            a.activation(out=ls[:], in_=ls[:], func=Act.Identity,
                         bias=rmax[:], scale=1.0)       # ls += rmax
            a.drain().then_inc(act_s, 1)
            stream_chunks(a, 1, start=7)

    return nc


def get_nc():
    if "nc" not in _CACHE:
        _CACHE["nc"] = _build_nc()
    return _CACHE["nc"]


def _pack_blob(cls_logits, iou_scores, target_ids, map_indices, map_ious,
               rand_vals):
    """Pack one core's small inputs into the [128, WBLOB] f32 blob.
    int32 fields are shipped as raw bits (.view), index rows are replicated
    across partitions (pure layout; no arithmetic on input values)."""
    blob = np.zeros((ROWS, WBLOB), dtype=np.float32)
    blob[:, COL_IOUSC] = iou_scores.reshape(ROWS)
    blob[:, COL_LG:COL_LG + C] = cls_logits.reshape(ROWS, C)
    blob[:, COL_TGT] = target_ids.reshape(ROWS).view(np.float32)
    blob[0:NJ, COL_IOU] = map_ious[:, 1:].reshape(NJ)
    blob[0:NJ, COL_RND] = rand_vals[:, 1:].reshape(NJ)
    blob[:, COL_IOTA] = np.arange(ROWS, dtype=np.float32)
    blob[:, COL_IOTA2D:COL_IOTA2D + ROWS] = np.arange(ROWS, dtype=np.float32)
    blob[0:NJ, COL_PJC] = map_indices[:, 0, 1:].reshape(NJ).view(np.float32)
    blob[0:NJ, COL_GJC] = map_indices[:, 1, 1:].reshape(NJ).view(np.float32)
    blob[0:NJ, COL_BADJ] = np.repeat(np.arange(BPC, dtype=np.float32) * CH, K)
    return blob


def make_in_maps(cls_logits, iou_scores, target_ids, map_indices, map_ious,
                 pred_mask_prob, rand_vals):
    """Shard full inputs into per-core input maps (layout/packing only)."""
    cls_logits = np.ascontiguousarray(np.asarray(cls_logits, dtype=np.float32))
    iou_scores = np.ascontiguousarray(np.asarray(iou_scores, dtype=np.float32))
    target_ids = np.ascontiguousarray(np.asarray(target_ids, dtype=np.int32))
    map_indices = np.ascontiguousarray(np.asarray(map_indices, dtype=np.int32))
    map_ious = np.ascontiguousarray(np.asarray(map_ious, dtype=np.float32))
    pred_mask_prob = np.ascontiguousarray(
        np.asarray(pred_mask_prob, dtype=np.float32))
    rand_vals = np.ascontiguousarray(np.asarray(rand_vals, dtype=np.float32))

    in_maps = []
    for c in range(N_CORES):
        b0, b1 = c * BPC, (c + 1) * BPC
        in_maps.append({
            "pmp": pred_mask_prob[b0:b1].reshape(ROWS, SPATIAL),
            "blob": _pack_blob(cls_logits[b0:b1], iou_scores[b0:b1],
                               target_ids[b0:b1], map_indices[b0:b1],
                               map_ious[b0:b1], rand_vals[b0:b1]),
        })
    return in_maps


def combine(parts):
    """parts: per-core (1,3) partial sums -> (iou_loss, cls_loss)."""
    s = np.stack([np.asarray(p, dtype=np.float32) for p in parts])
    s = s.reshape(-1, 3).sum(axis=0)
    wsum = s[2] + np.float32(1e-4)
    return np.asarray([s[0] / wsum, s[1] / wsum], dtype=np.float32)


def kernel(**inputs) -> np.ndarray:
    from concourse.bass_utils import run_bass_kernel_spmd

    nc = get_nc()
    in_maps = make_in_maps(**inputs)
    r = run_bass_kernel_spmd(nc, in_maps, core_ids=list(range(N_CORES)))
    parts = [r.results[c]["out"] for c in range(N_CORES)]
    return combine(parts)
